# revision 2
# baseline (speedup 1.0000x reference)
"""Butterfly transform kernel for Trainium2 (8 NeuronCores, SPMD data parallel).

Math: the 12 butterfly layers all pair the SAME adjacent columns (2n, 2n+1),
so they fold (host-side, in f64) into ONE per-pair 2x2 matrix
M[n] = W[0,n] @ ... @ W[11,n], and the kernel is a single memory-bound pass:
    y[:, 2n]   = x[:, 2n] * M[n,0,0] + x[:, 2n+1] * M[n,1,0]
    y[:, 2n+1] = x[:, 2n] * M[n,0,1] + x[:, 2n+1] * M[n,1,1]

This version halves the HBM traffic of the f32 DVE kernel (103.4 us) by
running the device pass entirely in bf16 (host converts in/out; rel err
~2e-3, far under the 2e-2 gate) and moves the math to the otherwise-idle
PE array:

  * Host transposes x per core to feature-major [4096, 1024] and swizzles it
    so each SBUF supertile [128, 8192] holds 8 feature-blocks of 128 features
    on the partition dim x 1024 batch columns, 16 KB/partition contiguous
    per DMA (the shape that measured ~340 GB/s).
  * The folded 2x2 weights become 32 block-diagonal 128x128 bf16 matrices
    (wb [128, 32*128]); block b's output is lhsT.T @ rhs on the PE
    (2 matmuls of N=512 into one PSUM f32 tile [128, 1024]).
  * ACT (even blocks) and DVE (odd blocks) copy PSUM f32 -> SBUF bf16
    (the downcast), and SWDGE stores the bf16 supertile.

Per-core per-pass: 8 MiB read + 8 MiB write = 16 MiB at the ~358 GB/s
HBM-per-core limit -> ~47 us roofline. PE ~17 us, ACT/DVE ~16/19 us all
hide underneath.
"""

import sys
import numpy as np

if "/opt/trn_rl_repo" not in sys.path:
    sys.path.insert(0, "/opt/trn_rl_repo")

BATCH = 8192
SIZE = 4096
LOG_N = 12
HALF = SIZE // 2  # 2048
N_CORES = 8
ROWS_PER_CORE = BATCH // N_CORES  # 1024
P = 128  # SBUF partitions
N_BLOCKS = SIZE // P  # 32 feature blocks per core
BLOCKS_PER_SUPER = 8
N_SUPER = N_BLOCKS // BLOCKS_PER_SUPER  # 4 supertiles per pass
SUPER_COLS = BLOCKS_PER_SUPER * ROWS_PER_CORE  # 8192 bf16 = 16 KB/partition

_CACHE = {}


def _np_bf16():
    import ml_dtypes

    return ml_dtypes.bfloat16


def _build_program(
    repeats: int = 1,
    mode: str = "mm",
    xio_bufs: int = 3,
    yio_bufs: int = 3,
    psum_bufs: int = 4,
):
    import concourse.bass as bass
    import concourse.bacc as bacc
    import concourse.mybir as mybir
    from concourse import tile
    from contextlib import ExitStack

    f32 = mybir.dt.float32
    bf16 = mybir.dt.bfloat16
    nc = bacc.Bacc(None, num_swdge_queues=4)

    x_in = nc.dram_tensor("x", [N_SUPER * P, SUPER_COLS], bf16, kind="ExternalInput")
    w_in = nc.dram_tensor("wb", [P, N_BLOCKS * P], bf16, kind="ExternalInput")
    y_out = nc.dram_tensor("y", [N_SUPER * P, SUPER_COLS], bf16, kind="ExternalOutput")

    with tile.TileContext(nc) as tc, ExitStack() as ctx:
        const = ctx.enter_context(tc.tile_pool(name="const", bufs=1))
        xio = ctx.enter_context(tc.tile_pool(name="xio", bufs=xio_bufs))
        yio = ctx.enter_context(tc.tile_pool(name="yio", bufs=yio_bufs))
        psum = ctx.enter_context(tc.tile_pool(name="psum", bufs=psum_bufs, space="PSUM"))

        wb = const.tile([P, N_BLOCKS * P], bf16)
        nc.sync.dma_start(wb[:], w_in[:])

        for t in range(N_SUPER * repeats):
            J = t % N_SUPER
            rows = slice(J * P, (J + 1) * P)
            xt = xio.tile([P, SUPER_COLS], bf16, tag="xt")
            nc.sync.dma_start(xt[:], x_in[rows, :])

            if mode == "copy":
                # DMA roofline probe: no compute, store the loaded tile.
                nc.gpsimd.dma_start(y_out[rows, :], xt[:])
                continue

            yt = yio.tile([P, SUPER_COLS], bf16, tag="yt")
            for j in range(BLOCKS_PER_SUPER):
                b = J * BLOCKS_PER_SUPER + j
                lhsT = wb[:, b * P : (b + 1) * P]
                cols = slice(j * ROWS_PER_CORE, (j + 1) * ROWS_PER_CORE)
                pt = psum.tile([P, ROWS_PER_CORE], f32, tag="pt")
                for h in range(ROWS_PER_CORE // 512):
                    nc.tensor.matmul(
                        pt[:, h * 512 : (h + 1) * 512],
                        lhsT,
                        xt[:, j * ROWS_PER_CORE + h * 512 : j * ROWS_PER_CORE + (h + 1) * 512],
                        start=True,
                        stop=True,
                    )
                # PSUM f32 -> SBUF bf16 downcast; alternate engines so the
                # ~1 us/block copies split ACT/DVE and hide under DMA.
                if j % 2 == 0:
                    nc.scalar.copy(yt[:, cols], pt[:])
                else:
                    nc.vector.tensor_copy(yt[:, cols], pt[:])
            nc.gpsimd.dma_start(y_out[rows, :], yt[:])

    nc.compile()
    return nc


def _get_nc(repeats: int = 1, mode: str = "mm", **kw):
    key = ("nc", repeats, mode, tuple(sorted(kw.items())))
    if key not in _CACHE:
        _CACHE[key] = _build_program(repeats, mode, **kw)
    return _CACHE[key]


def fold_weights(W: np.ndarray) -> np.ndarray:
    """Fold the 12 stacked 2x2 layers (f64) and emit the 32 block-diagonal
    128x128 bf16 PE weights, packed as wb [128, 32*128]:
    wb[q, b*128 + i] = M[b*64 + q//2, q%2, i%2] if i//2 == q//2 else 0."""
    bf16 = _np_bf16()
    Wd = W.astype(np.float64)  # [12, HALF, 2, 2]
    M = Wd[0]
    for l in range(1, Wd.shape[0]):
        M = np.einsum("nij,njk->nik", M, Wd[l])
    M = M.astype(np.float32)  # [HALF, 2, 2]
    Mr = M.reshape(N_BLOCKS, P // 2, 2, 2)  # [32, 64, 2, 2]
    Wfull = np.zeros((N_BLOCKS, P // 2, 2, P // 2, 2), np.float32)
    m = np.arange(P // 2)
    for a in range(2):
        for c in range(2):
            Wfull[:, m, a, m, c] = Mr[:, :, a, c]
    Wdense = Wfull.reshape(N_BLOCKS, P, P)  # [32, 128 fin, 128 fout]
    wb = Wdense.transpose(1, 0, 2).reshape(P, N_BLOCKS * P)
    return np.ascontiguousarray(wb).astype(bf16)


def pack_x(x: np.ndarray) -> list[np.ndarray]:
    """Per-core bf16 device layout [N_SUPER*128, 8192]:
    x_dev[J*128 + p, j*1024 + c] = x[core*1024 + c, (J*8 + j)*128 + p]."""
    bf16 = _np_bf16()
    xb = np.ascontiguousarray(x, dtype=np.float32).astype(bf16)
    out = []
    for core in range(N_CORES):
        xc = xb[core * ROWS_PER_CORE : (core + 1) * ROWS_PER_CORE]  # [1024, 4096]
        xd = (
            xc.reshape(ROWS_PER_CORE, N_SUPER, BLOCKS_PER_SUPER, P)
            .transpose(1, 3, 2, 0)
            .reshape(N_SUPER * P, SUPER_COLS)
        )
        out.append(np.ascontiguousarray(xd))
    return out


def unpack_y(y_devs: list[np.ndarray]) -> np.ndarray:
    """Inverse of pack_x on the outputs; returns f32 [BATCH, SIZE]."""
    outs = []
    for yd in y_devs:
        yc = (
            np.asarray(yd)
            .reshape(N_SUPER, P, BLOCKS_PER_SUPER, ROWS_PER_CORE)
            .transpose(3, 0, 2, 1)
            .reshape(ROWS_PER_CORE, SIZE)
        )
        outs.append(yc.astype(np.float32))
    return np.concatenate(outs, axis=0)


def _run(x: np.ndarray, W: np.ndarray, **run_kwargs):
    """Shard, run on the 8 cores, gather. Returns (output, BassKernelResults)."""
    from concourse.bass_utils import run_bass_kernel_spmd

    assert x.shape == (BATCH, SIZE) and W.shape == (LOG_N, HALF, 2, 2)
    wb = fold_weights(np.asarray(W))
    xds = pack_x(np.asarray(x))

    nc = _get_nc()
    in_maps = [{"x": xds[c], "wb": wb} for c in range(N_CORES)]
    res = run_bass_kernel_spmd(nc, in_maps, core_ids=list(range(N_CORES)), **run_kwargs)
    out = unpack_y([res.results[c]["y"] for c in range(N_CORES)])
    return out, res


def kernel(x: np.ndarray, W: np.ndarray) -> np.ndarray:
    return _run(x, W)[0]


# revision 8
# speedup vs baseline: 1.2889x; 1.2889x over previous
"""Butterfly transform kernel for Trainium2 (8 NeuronCores, SPMD data parallel).

Math: the 12 butterfly layers all pair the SAME adjacent columns (2n, 2n+1),
so they fold (host-side, in f64) into ONE per-pair 2x2 matrix
M[n] = W[0,n] @ ... @ W[11,n], and the kernel is a single memory-bound pass:
    y[:, 2n]   = x[:, 2n] * M[n,0,0] + x[:, 2n+1] * M[n,1,0]
    y[:, 2n+1] = x[:, 2n] * M[n,0,1] + x[:, 2n+1] * M[n,1,1]

This version halves the HBM traffic of the f32 DVE kernel (103.4 us) by
running the device pass entirely in bf16 (host converts in/out; rel err
~2e-3, far under the 2e-2 gate) and moves the math to the otherwise-idle
PE array:

  * Host transposes x per core to feature-major [4096, 1024] and swizzles it
    so each SBUF supertile [128, 8192] holds 8 feature-blocks of 128 features
    on the partition dim x 1024 batch columns, 16 KB/partition contiguous
    per DMA (the shape that measured ~340 GB/s).
  * The folded 2x2 weights become 32 block-diagonal 128x128 bf16 matrices
    (wb [128, 32*128]); block b's output is lhsT.T @ rhs on the PE
    (2 matmuls of N=512 into one PSUM f32 tile [128, 1024]).
  * ACT (even blocks) and DVE (odd blocks) copy PSUM f32 -> SBUF bf16
    (the downcast), and SWDGE stores the bf16 supertile.

Per-core per-pass: 8 MiB read + 8 MiB write = 16 MiB at the ~358 GB/s
HBM-per-core limit -> ~47 us roofline. PE ~17 us, ACT/DVE ~16/19 us all
hide underneath.
"""

import sys
import numpy as np

if "/opt/trn_rl_repo" not in sys.path:
    sys.path.insert(0, "/opt/trn_rl_repo")

BATCH = 8192
SIZE = 4096
LOG_N = 12
HALF = SIZE // 2  # 2048
N_CORES = 8
ROWS_PER_CORE = BATCH // N_CORES  # 1024
P = 128  # SBUF partitions
N_BLOCKS = SIZE // P  # 32 feature blocks per core
BLOCKS_PER_SUPER = 8
N_SUPER = N_BLOCKS // BLOCKS_PER_SUPER  # 4 supertiles per pass
SUPER_COLS = BLOCKS_PER_SUPER * ROWS_PER_CORE  # 8192 bf16 = 16 KB/partition

_CACHE = {}


def _np_bf16():
    import ml_dtypes

    return ml_dtypes.bfloat16


def _build_program(
    repeats: int = 1,
    mode: str = "mm",
    xio_bufs: int = 3,
    yio_bufs: int = 3,
    psum_bufs: int = 4,
    bps: int = BLOCKS_PER_SUPER,
    act_per_super: int | None = None,
):
    import concourse.bass as bass
    import concourse.bacc as bacc
    import concourse.mybir as mybir
    from concourse import tile
    from contextlib import ExitStack

    f32 = mybir.dt.float32
    bf16 = mybir.dt.bfloat16
    nc = bacc.Bacc(None, num_swdge_queues=4)

    n_super = N_BLOCKS // bps
    super_cols = bps * ROWS_PER_CORE
    if act_per_super is None:
        act_per_super = bps // 2  # ACT copies the first half of blocks, DVE the rest

    x_in = nc.dram_tensor("x", [n_super * P, super_cols], bf16, kind="ExternalInput")
    w_in = nc.dram_tensor("wb", [P, N_BLOCKS * P], bf16, kind="ExternalInput")
    y_out = nc.dram_tensor("y", [n_super * P, super_cols], bf16, kind="ExternalOutput")

    with tile.TileContext(nc) as tc, ExitStack() as ctx:
        const = ctx.enter_context(tc.tile_pool(name="const", bufs=1))
        xio = ctx.enter_context(tc.tile_pool(name="xio", bufs=xio_bufs))
        yio = ctx.enter_context(tc.tile_pool(name="yio", bufs=yio_bufs))
        psum = ctx.enter_context(tc.tile_pool(name="psum", bufs=psum_bufs, space="PSUM"))

        wb = const.tile([P, N_BLOCKS * P], bf16)
        nc.sync.dma_start(wb[:], w_in[:])

        if mode == "peonly":
            # PE-throughput probe: one resident tile, pure LDW+MM stream,
            # no DMA/copy dependencies. 64 MMs per "pass".
            xt0 = const.tile([P, super_cols], bf16)
            nc.sync.dma_start(xt0[:], x_in[0:P, :])
            for t in range(n_super * repeats):
                for j in range(bps):
                    b = (t % n_super) * bps + j
                    lhsT = wb[:, b * P : (b + 1) * P]
                    pt = psum.tile([P, ROWS_PER_CORE], f32, tag="pt")
                    for h in range(ROWS_PER_CORE // 512):
                        nc.tensor.matmul(
                            pt[:, h * 512 : (h + 1) * 512],
                            lhsT,
                            xt0[:, j * ROWS_PER_CORE + h * 512 : j * ROWS_PER_CORE + (h + 1) * 512],
                            start=True,
                            stop=True,
                        )
            nc.gpsimd.dma_start(y_out[0:P, :], xt0[:])
            nc.compile()
            return nc

        for t in range(n_super * repeats):
            J = t % n_super
            rows = slice(J * P, (J + 1) * P)
            xt = xio.tile([P, super_cols], bf16, tag="xt")
            nc.sync.dma_start(xt[:], x_in[rows, :])

            if mode == "copy":
                # DMA roofline probe: no compute, store the loaded tile.
                nc.gpsimd.dma_start(y_out[rows, :], xt[:])
                continue

            yt = yio.tile([P, super_cols], bf16, tag="yt")
            for j in range(bps):
                b = J * bps + j
                lhsT = wb[:, b * P : (b + 1) * P]
                cols = slice(j * ROWS_PER_CORE, (j + 1) * ROWS_PER_CORE)
                pt = psum.tile([P, ROWS_PER_CORE], f32, tag="pt")
                for h in range(ROWS_PER_CORE // 512):
                    nc.tensor.matmul(
                        pt[:, h * 512 : (h + 1) * 512],
                        lhsT,
                        xt[:, j * ROWS_PER_CORE + h * 512 : j * ROWS_PER_CORE + (h + 1) * 512],
                        start=True,
                        stop=True,
                    )
                if mode == "nocopy":
                    continue
                # PSUM f32 -> SBUF bf16 downcast; split ACT/DVE so the
                # ~1 us/block copies hide under DMA.
                if j < act_per_super:
                    nc.scalar.copy(yt[:, cols], pt[:])
                else:
                    nc.vector.tensor_copy(yt[:, cols], pt[:])
            if mode == "nocopy":
                # PE-only probe: store the input tile (results discarded).
                nc.gpsimd.dma_start(y_out[rows, :], xt[:])
            else:
                nc.gpsimd.dma_start(y_out[rows, :], yt[:])

    nc.compile()
    return nc


def _build_fs(
    repeats: int = 1,
    mode: str = "fs",
    xio_bufs: int = 3,
    yio_bufs: int = 3,
    psum_cols: int = 2048,
    act_share: tuple[int, int] = (9, 16),
):
    """Feature-sharded variant: each core owns 512 features x all 8192 batch.

    Per supertile [128 features, 8192 batch]: ONE weight block reused by 16
    matmuls of N=512 — a contiguous same-weight PE stream (1 useful LDW), vs
    32 weight switches per pass in the batch-sharded layout.  Copies
    PSUM f32 -> SBUF bf16 are split ACT/DVE by act_share.
    """
    import concourse.bacc as bacc
    import concourse.mybir as mybir
    from concourse import tile
    from contextlib import ExitStack

    f32 = mybir.dt.float32
    bf16 = mybir.dt.bfloat16
    nc = bacc.Bacc(None, num_swdge_queues=4)

    n_super = 4  # 4 feature blocks of 128 per core
    cols = BATCH  # 8192 batch columns
    x_in = nc.dram_tensor("x", [n_super * P, cols], bf16, kind="ExternalInput")
    w_in = nc.dram_tensor("wb", [P, n_super * P], bf16, kind="ExternalInput")
    y_out = nc.dram_tensor("y", [n_super * P, cols], bf16, kind="ExternalOutput")

    n_groups = cols // psum_cols
    psum_bufs = (8 * 512) // psum_cols  # use all 8 PSUM banks

    # weighted ACT/DVE round-robin for the copy chunks
    a_num, a_den = act_share

    def use_act(c):
        return (c + 1) * a_num // a_den > c * a_num // a_den

    with tile.TileContext(nc) as tc, ExitStack() as ctx:
        const = ctx.enter_context(tc.tile_pool(name="const", bufs=1))
        xio = ctx.enter_context(tc.tile_pool(name="xio", bufs=xio_bufs))
        yio = ctx.enter_context(tc.tile_pool(name="yio", bufs=yio_bufs))
        psum = ctx.enter_context(tc.tile_pool(name="psum", bufs=psum_bufs, space="PSUM"))

        wb = const.tile([P, n_super * P], bf16)
        nc.sync.dma_start(wb[:], w_in[:])

        c = 0
        for t in range(n_super * repeats):
            J = t % n_super
            rows = slice(J * P, (J + 1) * P)
            xt = xio.tile([P, cols], bf16, tag="xt")
            nc.sync.dma_start(xt[:], x_in[rows, :])
            yt = yio.tile([P, cols], bf16, tag="yt")
            lhsT = wb[:, J * P : (J + 1) * P]
            for g in range(n_groups):
                pt = psum.tile([P, psum_cols], f32, tag="pt")
                for h in range(psum_cols // 512):
                    o = g * psum_cols + h * 512
                    nc.tensor.matmul(
                        pt[:, h * 512 : (h + 1) * 512],
                        lhsT,
                        xt[:, o : o + 512],
                        start=True,
                        stop=True,
                    )
                chunk = slice(g * psum_cols, (g + 1) * psum_cols)
                if use_act(c):
                    nc.scalar.copy(yt[:, chunk], pt[:])
                else:
                    nc.vector.tensor_copy(yt[:, chunk], pt[:])
                c += 1
            nc.gpsimd.dma_start(y_out[rows, :], yt[:])

    nc.compile()
    return nc


def _get_nc(repeats: int = 1, mode: str = "mm", **kw):
    key = ("nc", repeats, mode, tuple(sorted(kw.items())))
    if key not in _CACHE:
        if mode.startswith("fs"):
            _CACHE[key] = _build_fs(repeats, mode, **kw)
        else:
            _CACHE[key] = _build_program(repeats, mode, **kw)
    return _CACHE[key]


def fold_weights(W: np.ndarray) -> np.ndarray:
    """Fold the 12 stacked 2x2 layers (f64) and emit the 32 block-diagonal
    128x128 bf16 PE weights, packed as wb [128, 32*128]:
    wb[q, b*128 + i] = M[b*64 + q//2, q%2, i%2] if i//2 == q//2 else 0."""
    bf16 = _np_bf16()
    Wd = W.astype(np.float64)  # [12, HALF, 2, 2]
    M = Wd[0]
    for l in range(1, Wd.shape[0]):
        M = np.einsum("nij,njk->nik", M, Wd[l])
    M = M.astype(np.float32)  # [HALF, 2, 2]
    Mr = M.reshape(N_BLOCKS, P // 2, 2, 2)  # [32, 64, 2, 2]
    Wfull = np.zeros((N_BLOCKS, P // 2, 2, P // 2, 2), np.float32)
    m = np.arange(P // 2)
    for a in range(2):
        for c in range(2):
            Wfull[:, m, a, m, c] = Mr[:, :, a, c]
    Wdense = Wfull.reshape(N_BLOCKS, P, P)  # [32, 128 fin, 128 fout]
    wb = Wdense.transpose(1, 0, 2).reshape(P, N_BLOCKS * P)
    return np.ascontiguousarray(wb).astype(bf16)


def pack_x(x: np.ndarray) -> list[np.ndarray]:
    """Per-core bf16 device layout [N_SUPER*128, 8192]:
    x_dev[J*128 + p, j*1024 + c] = x[core*1024 + c, (J*8 + j)*128 + p]."""
    bf16 = _np_bf16()
    xb = np.ascontiguousarray(x, dtype=np.float32).astype(bf16)
    out = []
    for core in range(N_CORES):
        xc = xb[core * ROWS_PER_CORE : (core + 1) * ROWS_PER_CORE]  # [1024, 4096]
        xd = (
            xc.reshape(ROWS_PER_CORE, N_SUPER, BLOCKS_PER_SUPER, P)
            .transpose(1, 3, 2, 0)
            .reshape(N_SUPER * P, SUPER_COLS)
        )
        out.append(np.ascontiguousarray(xd))
    return out


def unpack_y(y_devs: list[np.ndarray]) -> np.ndarray:
    """Inverse of pack_x on the outputs; returns f32 [BATCH, SIZE]."""
    outs = []
    for yd in y_devs:
        yc = (
            np.asarray(yd)
            .reshape(N_SUPER, P, BLOCKS_PER_SUPER, ROWS_PER_CORE)
            .transpose(3, 0, 2, 1)
            .reshape(ROWS_PER_CORE, SIZE)
        )
        outs.append(yc.astype(np.float32))
    return np.concatenate(outs, axis=0)


def _fold_dense(W: np.ndarray) -> np.ndarray:
    """Fold the 12 layers (f64) into dense block-diagonal [32, 128, 128] f32."""
    Wd = W.astype(np.float64)
    M = Wd[0]
    for l in range(1, Wd.shape[0]):
        M = np.einsum("nij,njk->nik", M, Wd[l])
    M = M.astype(np.float32)
    Mr = M.reshape(N_BLOCKS, P // 2, 2, 2)
    Wfull = np.zeros((N_BLOCKS, P // 2, 2, P // 2, 2), np.float32)
    m = np.arange(P // 2)
    for a in range(2):
        for c in range(2):
            Wfull[:, m, a, m, c] = Mr[:, :, a, c]
    return Wfull.reshape(N_BLOCKS, P, P)


def build_in_maps_fs(x: np.ndarray, W: np.ndarray) -> list[dict]:
    """Feature-sharded per-core inputs: x slab [512, 8192] (features major),
    per-core weights [128, 4*128]."""
    bf16 = _np_bf16()
    Wdense = _fold_dense(np.asarray(W))
    xb = np.ascontiguousarray(x, dtype=np.float32).astype(bf16)  # [8192, 4096]
    in_maps = []
    fpc = SIZE // N_CORES  # 512 features per core
    bpc = fpc // P  # 4 blocks per core
    for core in range(N_CORES):
        xs = np.ascontiguousarray(xb[:, core * fpc : (core + 1) * fpc].T)
        wc = (
            Wdense[core * bpc : (core + 1) * bpc]
            .transpose(1, 0, 2)
            .reshape(P, bpc * P)
            .astype(bf16)
        )
        in_maps.append({"x": xs, "wb": np.ascontiguousarray(wc)})
    return in_maps


def unpack_y_fs(y_devs: list[np.ndarray]) -> np.ndarray:
    out = np.empty((BATCH, SIZE), np.float32)
    fpc = SIZE // N_CORES
    for core, yd in enumerate(y_devs):
        out[:, core * fpc : (core + 1) * fpc] = np.asarray(yd).T.astype(np.float32)
    return out


def _run(x: np.ndarray, W: np.ndarray, **run_kwargs):
    """Shard, run on the 8 cores, gather. Returns (output, BassKernelResults)."""
    from concourse.bass_utils import run_bass_kernel_spmd

    assert x.shape == (BATCH, SIZE) and W.shape == (LOG_N, HALF, 2, 2)
    in_maps = build_in_maps_fs(np.asarray(x), np.asarray(W))
    nc = _get_nc(mode="fs")
    res = run_bass_kernel_spmd(nc, in_maps, core_ids=list(range(N_CORES)), **run_kwargs)
    out = unpack_y_fs([res.results[c]["y"] for c in range(N_CORES)])
    return out, res


def kernel(x: np.ndarray, W: np.ndarray) -> np.ndarray:
    return _run(x, W)[0]


# revision 11
# speedup vs baseline: 2.5365x; 1.9680x over previous
"""Butterfly transform kernel for Trainium2 (8 NeuronCores, SPMD data parallel).

Math: the 12 butterfly layers all pair the SAME adjacent columns (2n, 2n+1),
so they fold (host-side, in f64) into ONE per-pair 2x2 matrix
M[n] = W[0,n] @ ... @ W[11,n], and the kernel is a single memory-bound pass:
    y[:, 2n]   = x[:, 2n] * M[n,0,0] + x[:, 2n+1] * M[n,1,0]
    y[:, 2n+1] = x[:, 2n] * M[n,0,1] + x[:, 2n+1] * M[n,1,1]

This version halves the HBM traffic of the f32 DVE kernel (103.4 us) by
running the device pass entirely in bf16 (host converts in/out; rel err
~2e-3, far under the 2e-2 gate) and moves the math to the otherwise-idle
PE array:

  * Host transposes x per core to feature-major [4096, 1024] and swizzles it
    so each SBUF supertile [128, 8192] holds 8 feature-blocks of 128 features
    on the partition dim x 1024 batch columns, 16 KB/partition contiguous
    per DMA (the shape that measured ~340 GB/s).
  * The folded 2x2 weights become 32 block-diagonal 128x128 bf16 matrices
    (wb [128, 32*128]); block b's output is lhsT.T @ rhs on the PE
    (2 matmuls of N=512 into one PSUM f32 tile [128, 1024]).
  * ACT (even blocks) and DVE (odd blocks) copy PSUM f32 -> SBUF bf16
    (the downcast), and SWDGE stores the bf16 supertile.

Per-core per-pass: 8 MiB read + 8 MiB write = 16 MiB at the ~358 GB/s
HBM-per-core limit -> ~47 us roofline. PE ~17 us, ACT/DVE ~16/19 us all
hide underneath.
"""

import sys
import numpy as np

if "/opt/trn_rl_repo" not in sys.path:
    sys.path.insert(0, "/opt/trn_rl_repo")

BATCH = 8192
SIZE = 4096
LOG_N = 12
HALF = SIZE // 2  # 2048
N_CORES = 8
ROWS_PER_CORE = BATCH // N_CORES  # 1024
P = 128  # SBUF partitions
N_BLOCKS = SIZE // P  # 32 feature blocks per core
BLOCKS_PER_SUPER = 8
N_SUPER = N_BLOCKS // BLOCKS_PER_SUPER  # 4 supertiles per pass
SUPER_COLS = BLOCKS_PER_SUPER * ROWS_PER_CORE  # 8192 bf16 = 16 KB/partition

_CACHE = {}


def _np_bf16():
    import ml_dtypes

    return ml_dtypes.bfloat16


def _build_program(
    repeats: int = 1,
    mode: str = "mm",
    xio_bufs: int = 3,
    yio_bufs: int = 3,
    psum_bufs: int = 4,
    bps: int = BLOCKS_PER_SUPER,
    act_per_super: int | None = None,
):
    import concourse.bass as bass
    import concourse.bacc as bacc
    import concourse.mybir as mybir
    from concourse import tile
    from contextlib import ExitStack

    f32 = mybir.dt.float32
    bf16 = mybir.dt.bfloat16
    nc = bacc.Bacc(None, num_swdge_queues=4)

    n_super = N_BLOCKS // bps
    super_cols = bps * ROWS_PER_CORE
    if act_per_super is None:
        act_per_super = bps // 2  # ACT copies the first half of blocks, DVE the rest

    x_in = nc.dram_tensor("x", [n_super * P, super_cols], bf16, kind="ExternalInput")
    w_in = nc.dram_tensor("wb", [P, N_BLOCKS * P], bf16, kind="ExternalInput")
    y_out = nc.dram_tensor("y", [n_super * P, super_cols], bf16, kind="ExternalOutput")

    with tile.TileContext(nc) as tc, ExitStack() as ctx:
        const = ctx.enter_context(tc.tile_pool(name="const", bufs=1))
        xio = ctx.enter_context(tc.tile_pool(name="xio", bufs=xio_bufs))
        yio = ctx.enter_context(tc.tile_pool(name="yio", bufs=yio_bufs))
        psum = ctx.enter_context(tc.tile_pool(name="psum", bufs=psum_bufs, space="PSUM"))

        wb = const.tile([P, N_BLOCKS * P], bf16)
        nc.sync.dma_start(wb[:], w_in[:])

        if mode == "peonly":
            # PE-throughput probe: one resident tile, pure LDW+MM stream,
            # no DMA/copy dependencies. 64 MMs per "pass".
            xt0 = const.tile([P, super_cols], bf16)
            nc.sync.dma_start(xt0[:], x_in[0:P, :])
            for t in range(n_super * repeats):
                for j in range(bps):
                    b = (t % n_super) * bps + j
                    lhsT = wb[:, b * P : (b + 1) * P]
                    pt = psum.tile([P, ROWS_PER_CORE], f32, tag="pt")
                    for h in range(ROWS_PER_CORE // 512):
                        nc.tensor.matmul(
                            pt[:, h * 512 : (h + 1) * 512],
                            lhsT,
                            xt0[:, j * ROWS_PER_CORE + h * 512 : j * ROWS_PER_CORE + (h + 1) * 512],
                            start=True,
                            stop=True,
                        )
            nc.gpsimd.dma_start(y_out[0:P, :], xt0[:])
            nc.compile()
            return nc

        for t in range(n_super * repeats):
            J = t % n_super
            rows = slice(J * P, (J + 1) * P)
            xt = xio.tile([P, super_cols], bf16, tag="xt")
            nc.sync.dma_start(xt[:], x_in[rows, :])

            if mode == "copy":
                # DMA roofline probe: no compute, store the loaded tile.
                nc.gpsimd.dma_start(y_out[rows, :], xt[:])
                continue

            yt = yio.tile([P, super_cols], bf16, tag="yt")
            for j in range(bps):
                b = J * bps + j
                lhsT = wb[:, b * P : (b + 1) * P]
                cols = slice(j * ROWS_PER_CORE, (j + 1) * ROWS_PER_CORE)
                pt = psum.tile([P, ROWS_PER_CORE], f32, tag="pt")
                for h in range(ROWS_PER_CORE // 512):
                    nc.tensor.matmul(
                        pt[:, h * 512 : (h + 1) * 512],
                        lhsT,
                        xt[:, j * ROWS_PER_CORE + h * 512 : j * ROWS_PER_CORE + (h + 1) * 512],
                        start=True,
                        stop=True,
                    )
                if mode == "nocopy":
                    continue
                # PSUM f32 -> SBUF bf16 downcast; split ACT/DVE so the
                # ~1 us/block copies hide under DMA.
                if j < act_per_super:
                    nc.scalar.copy(yt[:, cols], pt[:])
                else:
                    nc.vector.tensor_copy(yt[:, cols], pt[:])
            if mode == "nocopy":
                # PE-only probe: store the input tile (results discarded).
                nc.gpsimd.dma_start(y_out[rows, :], xt[:])
            else:
                nc.gpsimd.dma_start(y_out[rows, :], yt[:])

    nc.compile()
    return nc


def _build_fs(
    repeats: int = 1,
    mode: str = "fs",
    xio_bufs: int = 3,
    yio_bufs: int = 3,
    psum_cols: int = 2048,
    act_share: tuple[int, int] = (9, 16),
    store_split: int = 0,
):
    """Feature-sharded variant: each core owns 512 features x all 8192 batch.

    Per supertile [128 features, 8192 batch]: ONE weight block reused by 16
    matmuls of N=512 — a contiguous same-weight PE stream (1 useful LDW), vs
    32 weight switches per pass in the batch-sharded layout.  Copies
    PSUM f32 -> SBUF bf16 are split ACT/DVE by act_share.
    """
    import concourse.bacc as bacc
    import concourse.mybir as mybir
    from concourse import tile
    from contextlib import ExitStack

    f32 = mybir.dt.float32
    bf16 = mybir.dt.bfloat16
    nc = bacc.Bacc(None, num_swdge_queues=4)

    n_super = 4  # 4 feature blocks of 128 per core
    cols = BATCH  # 8192 batch columns
    x_in = nc.dram_tensor("x", [n_super * P, cols], bf16, kind="ExternalInput")
    w_in = nc.dram_tensor("wb", [P, n_super * P], bf16, kind="ExternalInput")
    y_out = nc.dram_tensor("y", [n_super * P, cols], bf16, kind="ExternalOutput")

    n_groups = cols // psum_cols
    psum_bufs = (8 * 512) // psum_cols  # use all 8 PSUM banks

    # weighted ACT/DVE round-robin for the copy chunks
    a_num, a_den = act_share

    def use_act(c):
        return (c + 1) * a_num // a_den > c * a_num // a_den

    with tile.TileContext(nc) as tc, ExitStack() as ctx:
        const = ctx.enter_context(tc.tile_pool(name="const", bufs=1))
        xio = ctx.enter_context(tc.tile_pool(name="xio", bufs=xio_bufs))
        yio = ctx.enter_context(tc.tile_pool(name="yio", bufs=yio_bufs))
        psum = ctx.enter_context(tc.tile_pool(name="psum", bufs=psum_bufs, space="PSUM"))

        wb = const.tile([P, n_super * P], bf16)
        nc.sync.dma_start(wb[:], w_in[:])

        c = 0
        for t in range(n_super * repeats):
            J = t % n_super
            rows = slice(J * P, (J + 1) * P)
            xt = xio.tile([P, cols], bf16, tag="xt")
            nc.sync.dma_start(xt[:], x_in[rows, :])
            yt = yio.tile([P, cols], bf16, tag="yt")
            lhsT = wb[:, J * P : (J + 1) * P]
            for g in range(n_groups):
                pt = psum.tile([P, psum_cols], f32, tag="pt")
                for h in range(psum_cols // 512):
                    o = g * psum_cols + h * 512
                    nc.tensor.matmul(
                        pt[:, h * 512 : (h + 1) * 512],
                        lhsT,
                        xt[:, o : o + 512],
                        start=True,
                        stop=True,
                    )
                if mode == "fsnc":
                    c += 1
                    continue
                chunk = slice(g * psum_cols, (g + 1) * psum_cols)
                if use_act(c):
                    nc.scalar.copy(yt[:, chunk], pt[:])
                else:
                    nc.vector.tensor_copy(yt[:, chunk], pt[:])
                c += 1
                if store_split:
                    # store each chunk as soon as its copy lands
                    nc.gpsimd.dma_start(y_out[rows, chunk], yt[:, chunk])
            if mode == "fsnc":
                nc.gpsimd.dma_start(y_out[rows, :], xt[:])
            elif not store_split:
                nc.gpsimd.dma_start(y_out[rows, :], yt[:])

    nc.compile()
    return nc


def _get_nc(repeats: int = 1, mode: str = "mm", **kw):
    key = ("nc", repeats, mode, tuple(sorted(kw.items())))
    if key not in _CACHE:
        if mode.startswith("fs"):
            _CACHE[key] = _build_fs(repeats, mode, **kw)
        else:
            _CACHE[key] = _build_program(repeats, mode, **kw)
    return _CACHE[key]


def fold_weights(W: np.ndarray) -> np.ndarray:
    """Fold the 12 stacked 2x2 layers (f64) and emit the 32 block-diagonal
    128x128 bf16 PE weights, packed as wb [128, 32*128]:
    wb[q, b*128 + i] = M[b*64 + q//2, q%2, i%2] if i//2 == q//2 else 0."""
    bf16 = _np_bf16()
    Wd = W.astype(np.float64)  # [12, HALF, 2, 2]
    M = Wd[0]
    for l in range(1, Wd.shape[0]):
        M = np.einsum("nij,njk->nik", M, Wd[l])
    M = M.astype(np.float32)  # [HALF, 2, 2]
    Mr = M.reshape(N_BLOCKS, P // 2, 2, 2)  # [32, 64, 2, 2]
    Wfull = np.zeros((N_BLOCKS, P // 2, 2, P // 2, 2), np.float32)
    m = np.arange(P // 2)
    for a in range(2):
        for c in range(2):
            Wfull[:, m, a, m, c] = Mr[:, :, a, c]
    Wdense = Wfull.reshape(N_BLOCKS, P, P)  # [32, 128 fin, 128 fout]
    wb = Wdense.transpose(1, 0, 2).reshape(P, N_BLOCKS * P)
    return np.ascontiguousarray(wb).astype(bf16)


def pack_x(x: np.ndarray) -> list[np.ndarray]:
    """Per-core bf16 device layout [N_SUPER*128, 8192]:
    x_dev[J*128 + p, j*1024 + c] = x[core*1024 + c, (J*8 + j)*128 + p]."""
    bf16 = _np_bf16()
    xb = np.ascontiguousarray(x, dtype=np.float32).astype(bf16)
    out = []
    for core in range(N_CORES):
        xc = xb[core * ROWS_PER_CORE : (core + 1) * ROWS_PER_CORE]  # [1024, 4096]
        xd = (
            xc.reshape(ROWS_PER_CORE, N_SUPER, BLOCKS_PER_SUPER, P)
            .transpose(1, 3, 2, 0)
            .reshape(N_SUPER * P, SUPER_COLS)
        )
        out.append(np.ascontiguousarray(xd))
    return out


def unpack_y(y_devs: list[np.ndarray]) -> np.ndarray:
    """Inverse of pack_x on the outputs; returns f32 [BATCH, SIZE]."""
    outs = []
    for yd in y_devs:
        yc = (
            np.asarray(yd)
            .reshape(N_SUPER, P, BLOCKS_PER_SUPER, ROWS_PER_CORE)
            .transpose(3, 0, 2, 1)
            .reshape(ROWS_PER_CORE, SIZE)
        )
        outs.append(yc.astype(np.float32))
    return np.concatenate(outs, axis=0)


def _fold_dense(W: np.ndarray) -> np.ndarray:
    """Fold the 12 layers (f64) into dense block-diagonal [32, 128, 128] f32."""
    Wd = W.astype(np.float64)
    M = Wd[0]
    for l in range(1, Wd.shape[0]):
        M = np.einsum("nij,njk->nik", M, Wd[l])
    M = M.astype(np.float32)
    Mr = M.reshape(N_BLOCKS, P // 2, 2, 2)
    Wfull = np.zeros((N_BLOCKS, P // 2, 2, P // 2, 2), np.float32)
    m = np.arange(P // 2)
    for a in range(2):
        for c in range(2):
            Wfull[:, m, a, m, c] = Mr[:, :, a, c]
    return Wfull.reshape(N_BLOCKS, P, P)


def build_in_maps_fs(x: np.ndarray, W: np.ndarray) -> list[dict]:
    """Feature-sharded per-core inputs: x slab [512, 8192] (features major),
    per-core weights [128, 4*128]."""
    bf16 = _np_bf16()
    Wdense = _fold_dense(np.asarray(W))
    xb = np.ascontiguousarray(x, dtype=np.float32).astype(bf16)  # [8192, 4096]
    in_maps = []
    fpc = SIZE // N_CORES  # 512 features per core
    bpc = fpc // P  # 4 blocks per core
    for core in range(N_CORES):
        xs = np.ascontiguousarray(xb[:, core * fpc : (core + 1) * fpc].T)
        wc = (
            Wdense[core * bpc : (core + 1) * bpc]
            .transpose(1, 0, 2)
            .reshape(P, bpc * P)
            .astype(bf16)
        )
        in_maps.append({"x": xs, "wb": np.ascontiguousarray(wc)})
    return in_maps


def unpack_y_fs(y_devs: list[np.ndarray]) -> np.ndarray:
    out = np.empty((BATCH, SIZE), np.float32)
    fpc = SIZE // N_CORES
    for core, yd in enumerate(y_devs):
        out[:, core * fpc : (core + 1) * fpc] = np.asarray(yd).T.astype(np.float32)
    return out


def _run(x: np.ndarray, W: np.ndarray, **run_kwargs):
    """Shard, run on the 8 cores, gather. Returns (output, BassKernelResults)."""
    from concourse.bass_utils import run_bass_kernel_spmd

    assert x.shape == (BATCH, SIZE) and W.shape == (LOG_N, HALF, 2, 2)
    in_maps = build_in_maps_fs(np.asarray(x), np.asarray(W))
    nc = _get_nc(mode="fs")
    res = run_bass_kernel_spmd(nc, in_maps, core_ids=list(range(N_CORES)), **run_kwargs)
    out = unpack_y_fs([res.results[c]["y"] for c in range(N_CORES)])
    return out, res


def kernel(x: np.ndarray, W: np.ndarray) -> np.ndarray:
    return _run(x, W)[0]
